# revision 32
# baseline (speedup 1.0000x reference)
"""Block-causal (frame-windowed) attention layer for Trainium2, 8-core SPMD.

Reference computation (B=4, T=2048, C=512, H=8, Dh=64, NPATCH=256):
  LayerNorm(x) -> qkv = xn @ w_qkv -> per-head attention with mask
  frame(i) >= frame(j), frame = idx // 256 -> out @ w_out + b_out

Sharding: core c handles batch c//2 and heads (c%2)*4 .. (c%2)*4+3.
Each core computes a partial y (its heads' contribution to out @ w_out);
the host sums the two partials per batch and adds b_out.

Key design points (v2):
 - frames are 256 wide and the mask is frame-aligned: no masking ops.
 - S^T layout [keys, q]; softmax normalizer via ones-column in V.
 - x is fp16 on the wire; LayerNorm apply on GpSimd; transposes are
   plain PE matmuls vs identity (fast + keeps HAM warm); PE warmup burst.
 - S^T matmuls of a head pair are row-packed (base partitions 0/64) so
   they run concurrently in distinct PE row groups.
 - exp: 3 of 4 heads on ScalarE (exact); head 3 uses a Schraudolph-style
   fp16 exp on VectorE (affine in log2-space, bitcast uint16->fp16).
   The 2^2 exponent offset cancels in the softmax normalizer.
"""

import sys

sys.path.insert(0, "/opt/trn_rl_repo")

import numpy as np

import concourse.bacc as bacc
import concourse.bass as bass
import concourse.mybir as mybir
import concourse.tile as tile
from concourse.bass_utils import run_bass_kernel_spmd
from concourse.masks import make_identity

B, T, C = 4, 2048, 512
HEADS, DH = 8, 64
NPATCH = 256
EPS = 1e-5
N_CORES = 8
HPC = HEADS // 2          # heads per core = 4
QK_COLS = HPC * DH * 2    # 512 (q block + k block)
V_COLS = HPC * DH         # 256
NT = T // 128             # 16 token tiles
NF = T // NPATCH          # 8 frames
NCC = C // 128            # 4 contraction chunks

F32 = mybir.dt.float32
FP16 = mybir.dt.float16
U16 = mybir.dt.uint16
AF = mybir.ActivationFunctionType
ALU = mybir.AluOpType

# Schraudolph fp16 exp: bits = S*1024/ln2 + (15*1024 - 44.7 + 2048)
# (the +2048 = 2^2 scale keeps the affine form inside [0, 32768) for
# S in (-11.7, +10.4); the 4x scale cancels in the softmax normalizer)
EXP_A = 1024.0 / float(np.log(2.0))
EXP_B = 15 * 1024 - 44.7 + 2048.0

_cache = {}
_run_opts = {}      # test harness may set {"trace": True, ...}
_last_res = [None]  # last BassKernelResults, for profiling
import os
_NO_VEXP = bool(os.environ.get("NO_VEXP"))
_DEBUG = bool(os.environ.get("KDEBUG"))
_NO_PACK = bool(os.environ.get("NO_PACK"))    # ST matmuls full-width K=128
_OLD_T = bool(os.environ.get("OLD_T"))        # transpose-mode transposes
_OLD_LN = bool(os.environ.get("OLD_LN"))      # LN apply on vector
_OLD_VEVAC = bool(os.environ.get("OLD_VEVAC"))  # v evac on vector


def _build(with_qkv_bias: bool):
    nc = bacc.Bacc("TRN2", target_bir_lowering=False, debug=False,
                   num_devices=N_CORES)
    x_d = nc.dram_tensor("x", [T, C], FP16, kind="ExternalInput").ap()
    wqk_d = nc.dram_tensor("wqk", [C, QK_COLS], FP16, kind="ExternalInput").ap()
    wv_d = nc.dram_tensor("wv", [C, V_COLS], FP16, kind="ExternalInput").ap()
    wo_d = nc.dram_tensor("wo", [V_COLS, C], FP16, kind="ExternalInput").ap()
    if with_qkv_bias:
        bqk_d = nc.dram_tensor("bqk", [1, QK_COLS], F32, kind="ExternalInput").ap()
        bv_d = nc.dram_tensor("bv", [1, 2 * V_COLS], F32, kind="ExternalInput").ap()
    y_d = nc.dram_tensor("y", [T, C], FP16, kind="ExternalOutput").ap()
    dbg = None
    if _DEBUG:
        dbg = {
            "dbg_qkT": nc.dram_tensor("dbg_qkT", [128, NCC * T], FP16,
                                      kind="ExternalOutput").ap(),
            "dbg_v": nc.dram_tensor("dbg_v", [128, NT * HPC * (DH + 1)], FP16,
                                    kind="ExternalOutput").ap(),
            "dbg_oT": nc.dram_tensor("dbg_oT", [128, 2 * T], FP16,
                                     kind="ExternalOutput").ap(),
            "dbg_pv": nc.dram_tensor("dbg_pv", [65, 16 * 512], F32,
                                     kind="ExternalOutput").ap(),
            "dbg_rec": nc.dram_tensor("dbg_rec", [1, 16 * 512], F32,
                                      kind="ExternalOutput").ap(),
        }

    with tile.TileContext(nc) as tc:
        _emit(nc, tc, x_d, wqk_d, wv_d, wo_d, y_d,
              (bqk_d, bv_d) if with_qkv_bias else None, dbg)
    nc.compile()
    return nc


def _emit(nc, tc, x_d, wqk_d, wv_d, wo_d, y_d, biases, dbg=None):
    from contextlib import ExitStack
    ctx = ExitStack()
    with ctx:
        singles = ctx.enter_context(tc.tile_pool(name="singles", bufs=1))
        xnp = ctx.enter_context(tc.tile_pool(name="xnp", bufs=3))
        stats = ctx.enter_context(tc.tile_pool(name="stats", bufs=4))
        ptp = ctx.enter_context(tc.tile_pool(name="ptp", bufs=4))
        recips = ctx.enter_context(tc.tile_pool(name="recips", bufs=2))
        rrepp = ctx.enter_context(tc.tile_pool(name="rrepp", bufs=2))
        yp = ctx.enter_context(tc.tile_pool(name="yp", bufs=3))
        ps_mm = ctx.enter_context(tc.tile_pool(name="ps_mm", bufs=2, space="PSUM"))
        ps_st = ctx.enter_context(tc.tile_pool(name="ps_st", bufs=4, space="PSUM"))
        ps_pv = ctx.enter_context(tc.tile_pool(name="ps_pv", bufs=2, space="PSUM"))

        # ---- identity + constants first (gate the PE warmup burst) ----
        ident = singles.tile([128, 128], FP16)
        make_identity(nc, ident)
        warm_src = singles.tile([128, 512], FP16)
        nc.vector.memset(warm_src, 0.0)

        # ---- input DMA: x in chunks on two queues, weights on scalar ----
        xt_all = singles.tile([128, NT, C], FP16)
        x_r = x_d.rearrange("(n p) c -> p n c", p=128)
        nc.sync.dma_start(out=xt_all[:, 0:2, :], in_=x_r[:, 0:2, :])
        nc.gpsimd.dma_start(out=xt_all[:, 2:5, :], in_=x_r[:, 2:5, :])
        nc.sync.dma_start(out=xt_all[:, 5:10, :], in_=x_r[:, 5:10, :])
        nc.gpsimd.dma_start(out=xt_all[:, 10:16, :], in_=x_r[:, 10:16, :])

        wqk = singles.tile([128, NCC, QK_COLS], FP16)
        wv = singles.tile([128, NCC, V_COLS], FP16)
        wo = singles.tile([128, 2, C], FP16)
        nc.scalar.dma_start(
            out=wqk, in_=wqk_d.rearrange("(cc p) n -> p cc n", p=128))
        nc.scalar.dma_start(
            out=wv, in_=wv_d.rearrange("(cc p) n -> p cc n", p=128))
        nc.scalar.dma_start(
            out=wo, in_=wo_d.rearrange("(i p) n -> p i n", p=128))

        if biases is not None:
            bqk_d, bv_d = biases
            bqk_sb = singles.tile([128, NCC, 1], F32)
            nc.gpsimd.dma_start(
                out=bqk_sb, in_=bqk_d.rearrange("o (d p) -> p d o", p=128))
            # v bias duplicated (two token tiles share one evac): [128, 512]
            bv_sb = singles.tile([128, 2 * V_COLS], F32)
            nc.gpsimd.dma_start(out=bv_sb,
                                in_=bv_d.to_broadcast((128, 2 * V_COLS)))

        # ---- persistent tiles ----
        eps_t = singles.tile([128, 1], F32)
        nc.vector.memset(eps_t, EPS)

        xnT = singles.tile([128, NCC, T], FP16)      # [C dims, (cc, tok)]
        qkT = singles.tile([128, NCC, T], FP16)      # d0,d1 = q; d2,d3 = k
        v_all = singles.tile([128, NT, HPC, DH + 1], FP16)
        oT = singles.tile([128, 2, T], FP16)         # [inner dims, tok]

        ones_stage = singles.tile([128, NT * HPC], F32)
        nc.vector.memset(ones_stage, 1.0)
        nc.vector.tensor_copy(
            out=v_all[:, :, :, DH:DH + 1].rearrange("p t h o -> p (t h o)"),
            in_=ones_stage)

        # ---- PE warmup: dummy matmuls so HAM un-throttles before stage B ----
        for w in range(8):
            pw = ps_st.tile([128, 512], F32, tag="st")
            nc.tensor.matmul(pw, ident, warm_src, start=True, stop=True)

        # ---- stage A: LayerNorm + transpose into xnT ----
        for t in range(NT):
            xt = xt_all[:, t, :]
            st6 = stats.tile([128, 6], F32)
            nc.vector.bn_stats(out=st6, in_=xt)
            mv = stats.tile([128, 2], F32)
            nc.vector.bn_aggr(out=mv, in_=st6)
            rstd = stats.tile([128, 1], F32)
            nc.scalar.activation(out=rstd, in_=mv[:, 1:2], func=AF.Sqrt,
                                 bias=eps_t, scale=1.0)
            nc.vector.reciprocal(out=rstd, in_=rstd)
            xn = xnp.tile([128, C], FP16)
            # NOTE: gpsimd tensor_scalar with AP scalars is ~7.6us/op - keep
            # the LN apply on the vector engine.
            nc.vector.tensor_scalar(
                out=xn, in0=xt, scalar1=mv[:, 0:1], scalar2=rstd,
                op0=ALU.subtract, op1=ALU.mult)
            if _OLD_T:
                tp16 = ps_mm.tile([128, 512], FP16, tag="mm")
                for cc in range(NCC):
                    nc.tensor.transpose(
                        tp16[:, cc * 128:(cc + 1) * 128],
                        xn[:, cc * 128:(cc + 1) * 128], ident)
                nc.vector.tensor_copy(
                    out=xnT[:, :, t * 128:(t + 1) * 128],
                    in_=tp16.rearrange("p (cc q) -> p cc q", cc=NCC))
            else:
                tp = ps_mm.tile([128, 512], F32, tag="mm")
                for cc in range(NCC):
                    nc.tensor.matmul(
                        tp[:, cc * 128:(cc + 1) * 128],
                        xn[:, cc * 128:(cc + 1) * 128], ident,
                        start=True, stop=True)
                nc.scalar.activation(
                    out=xnT[:, :, t * 128:(t + 1) * 128],
                    in_=tp.rearrange("p (cc q) -> p cc q", cc=NCC),
                    func=AF.Copy)
            if t % 2 == 0:   # PE filler: keep HAM busy through stage A
                pw = ps_st.tile([128, 512], F32, tag="st")
                nc.tensor.matmul(pw, ident, warm_src, start=True, stop=True)

        # ---- stage B: qkT = w_qk^T @ xn^T ; v = xn @ w_v ----
        for n in range(4):           # token groups of 512
            for d in range(NCC):     # qk dim chunks
                mm = ps_mm.tile([128, 512], F32, tag="mm")
                for cc in range(NCC):
                    nc.tensor.matmul(
                        mm,
                        wqk[:, cc, d * 128:(d + 1) * 128],
                        xnT[:, cc, n * 512:(n + 1) * 512],
                        start=(cc == 0), stop=(cc == NCC - 1))
                if biases is not None:
                    nc.vector.tensor_scalar(
                        out=qkT[:, d, n * 512:(n + 1) * 512], in0=mm,
                        scalar1=bqk_sb[:, d, :], scalar2=None,
                        op0=ALU.add)
                else:
                    nc.vector.tensor_copy(
                        out=qkT[:, d, n * 512:(n + 1) * 512], in_=mm)
            for half in range(2):    # two token tiles per PSUM bank
                t0 = 4 * n + 2 * half
                mm = ps_mm.tile([128, 512], F32, tag="mm")
                for sub in range(2):
                    t = t0 + sub
                    for cc in range(NCC):
                        nc.tensor.matmul(
                            mm[:, sub * 256:(sub + 1) * 256],
                            xnT[:, cc, t * 128:(t + 1) * 128],
                            wv[:, cc, :],
                            start=(cc == 0), stop=(cc == NCC - 1))
                v_out = v_all[:, t0:t0 + 2, :, 0:DH]
                v_in = mm.rearrange("p (t h d) -> p t h d", t=2, h=HPC)
                if biases is not None:
                    nc.vector.tensor_tensor(
                        out=v_out, in0=v_in,
                        in1=bv_sb.rearrange("p (t h d) -> p t h d",
                                            t=2, h=HPC),
                        op=ALU.add)
                elif _OLD_VEVAC:
                    nc.vector.tensor_copy(out=v_out, in_=v_in)
                else:
                    nc.scalar.activation(out=v_out, in_=v_in, func=AF.Copy)

        # ---- stage C: attention per (frame, head pair); D: out-proj ----
        for f in range(NF):
            for p in range(2):       # head pair: heads (2p, 2p+1)
                dq, dk = p, 2 + p
                q_a = qkT[0:64, dq, f * 256:(f + 1) * 256]
                q_b = qkT[64:128, dq, f * 256:(f + 1) * 256]
                # both heads share one PSUM bank; a matmul's start=True
                # clears the WHOLE bank's has_written, so only head A's
                # first matmul carries start=True (head B's first write then
                # lands on already-cleared has_written and overwrites)
                pvt = ps_pv.tile([65, 512], F32, tag="pv")
                pv = (pvt[:, 0:256], pvt[:, 256:512])
                pts = []
                for g in range(f + 1):   # source frame (2 key chunks)
                    st_a = ps_st.tile([128, 512], F32, tag="st")
                    st_b = ps_st.tile([128, 512], F32, tag="st")
                    for j in range(2):
                        kc = 2 * g + j
                        ksl = slice(kc * 128, (kc + 1) * 128)
                        csl = slice(j * 256, (j + 1) * 256)
                        if _NO_PACK:
                            nc.tensor.matmul(st_a[:, csl],
                                             qkT[0:64, dk, ksl], q_a,
                                             start=True, stop=True,
                                             tile_position=(0, 0))
                            nc.tensor.matmul(st_b[:, csl],
                                             qkT[64:128, dk, ksl], q_b,
                                             start=True, stop=True,
                                             tile_position=(0, 0))
                        else:
                            nc.tensor.matmul(st_a[:, csl],
                                             qkT[0:64, dk, ksl], q_a,
                                             start=True, stop=True)
                            nc.tensor.matmul(st_b[:, csl],
                                             qkT[64:128, dk, ksl], q_b,
                                             start=True, stop=True)
                    pt_a = ptp.tile([128, 512], FP16)
                    pt_b = ptp.tile([128, 512], FP16)
                    nc.scalar.activation(out=pt_a, in_=st_a, func=AF.Exp)
                    if p == 1 and not _NO_VEXP:
                        # fast exp on DVE: affine in log2 space -> uint16
                        nc.vector.tensor_scalar(
                            out=pt_b.bitcast(U16), in0=st_b,
                            scalar1=EXP_A, scalar2=EXP_B,
                            op0=ALU.mult, op1=ALU.add)
                    else:
                        nc.scalar.activation(out=pt_b, in_=st_b, func=AF.Exp)
                    pts.append((pt_a, pt_b))
                    if g > 0:
                        _emit_pv(nc, pv, v_all, pts[g - 1], p, f, g - 1)
                _emit_pv(nc, pv, v_all, pts[f], p, f, f)
                # normalize: oT = pv[:64] / pv[64]
                ssum = recips.tile([1, 512], F32)
                nc.vector.tensor_copy(out=ssum, in_=pvt[64:65, :])
                rec = recips.tile([1, 512], F32)
                nc.vector.reciprocal_approx_fast(out=rec, in_=ssum)
                if dbg is not None:
                    fp = 2 * f + p
                    pv_sb = singles.tile([65, 16, 512], F32, tag="dbg_pv_sb")
                    nc.vector.tensor_copy(out=pv_sb[:, fp, :], in_=pvt)
                    nc.sync.dma_start(
                        out=dbg["dbg_pv"][:, fp * 512:(fp + 1) * 512],
                        in_=pv_sb[:, fp, :])
                    nc.sync.dma_start(
                        out=dbg["dbg_rec"][:, fp * 512:(fp + 1) * 512],
                        in_=rec)
                rrep = rrepp.tile([64, 512], F32)
                nc.gpsimd.partition_broadcast(rrep, rec)
                fsl = slice(f * 256, (f + 1) * 256)
                nc.vector.tensor_tensor(
                    out=oT[0:64, p, fsl], in0=pvt[0:64, 0:256],
                    in1=rrep[:, 0:256], op=ALU.mult)
                nc.vector.tensor_tensor(
                    out=oT[64:128, p, fsl], in0=pvt[0:64, 256:512],
                    in1=rrep[:, 256:512], op=ALU.mult)
            # out-projection for this frame's two token tiles
            for t in (2 * f, 2 * f + 1):
                ym = ps_mm.tile([128, 512], F32, tag="mm")
                for i in range(2):
                    nc.tensor.matmul(
                        ym, oT[:, i, t * 128:(t + 1) * 128], wo[:, i, :],
                        start=(i == 0), stop=(i == 1))
                ysb = yp.tile([128, C], FP16)
                nc.vector.tensor_copy(out=ysb, in_=ym)
                nc.gpsimd.dma_start(
                    out=y_d[t * 128:(t + 1) * 128, :], in_=ysb)
        if dbg is not None:
            nc.sync.dma_start(
                out=dbg["dbg_qkT"],
                in_=qkT.rearrange("p cc t -> p (cc t)"))
            nc.sync.dma_start(
                out=dbg["dbg_v"],
                in_=v_all.rearrange("p t h d -> p (t h d)"))
            nc.sync.dma_start(
                out=dbg["dbg_oT"], in_=oT.rearrange("p i t -> p (i t)"))


def _emit_pv(nc, pv, v_all, pt_pair, p, f, g):
    pv_a, pv_b = pv
    pt_a, pt_b = pt_pair
    last = f
    for j in range(2):
        kc = 2 * g + j
        csl = slice(j * 256, (j + 1) * 256)
        # start=True ONLY on head A's first matmul: it clears the whole
        # bank's has_written, covering head B's region too.
        nc.tensor.matmul(pv_a, v_all[:, kc, 2 * p, :], pt_a[:, csl],
                         start=(kc == 0), stop=(kc == 2 * last + 1),
                         skip_group_check=True)
        nc.tensor.matmul(pv_b, v_all[:, kc, 2 * p + 1, :], pt_b[:, csl],
                         start=False, stop=(kc == 2 * last + 1),
                         skip_group_check=True)


def kernel(x, ln_gamma, ln_beta, w_qkv, w_out, b_out, mask):
    x = np.asarray(x, dtype=np.float32)
    ln_gamma = np.asarray(ln_gamma, dtype=np.float32)
    ln_beta = np.asarray(ln_beta, dtype=np.float32)
    w_qkv = np.asarray(w_qkv, dtype=np.float32)
    w_out = np.asarray(w_out, dtype=np.float32)
    b_out = np.asarray(b_out, dtype=np.float32)

    inner = HEADS * DH
    wq_all = w_qkv[:, 0:inner] * ln_gamma[:, None]
    wk_all = w_qkv[:, inner:2 * inner] * ln_gamma[:, None]
    wv_all = w_qkv[:, 2 * inner:3 * inner] * ln_gamma[:, None]
    scale = DH ** -0.5
    # beta contribution to q/k/v (exact: qkv = ln(x)@(gamma*W) + beta@W)
    bq_all = ln_beta @ w_qkv[:, 0:inner]
    bk_all = ln_beta @ w_qkv[:, inner:2 * inner]
    bv_all = ln_beta @ w_qkv[:, 2 * inner:3 * inner]
    with_bias = bool(
        np.abs(bq_all).max() > 0 or np.abs(bk_all).max() > 0
        or np.abs(bv_all).max() > 0)

    key = ("prog", with_bias, _NO_VEXP, _NO_PACK, _OLD_T, _OLD_LN, _OLD_VEVAC,
           _DEBUG)
    if key not in _cache:
        _cache[key] = _build(with_bias)
    nc = _cache[key]

    in_maps = []
    for c in range(N_CORES):
        b = c // 2
        h0 = (c % 2) * HPC
        cols = slice(h0 * DH, (h0 + HPC) * DH)
        wqk_c = np.concatenate([wq_all[:, cols] * scale, wk_all[:, cols]],
                               axis=1)
        m = {
            "x": np.ascontiguousarray(x[b].astype(np.float16)),
            "wqk": np.ascontiguousarray(wqk_c.astype(np.float16)),
            "wv": np.ascontiguousarray(wv_all[:, cols].astype(np.float16)),
            "wo": np.ascontiguousarray(w_out[cols, :].astype(np.float16)),
        }
        if with_bias:
            bqk_c = np.concatenate([bq_all[cols] * scale, bk_all[cols]])
            m["bqk"] = np.ascontiguousarray(bqk_c[None, :])
            m["bv"] = np.ascontiguousarray(
                np.tile(bv_all[cols], 2)[None, :])
        in_maps.append(m)

    res = run_bass_kernel_spmd(nc, in_maps, core_ids=list(range(N_CORES)),
                               **_run_opts)
    _last_res[0] = res
    y = np.empty((B, T, C), dtype=np.float32)
    for b in range(B):
        y[b] = (res.results[2 * b]["y"].astype(np.float32)
                + res.results[2 * b + 1]["y"].astype(np.float32) + b_out)
    return y


# revision 36
# speedup vs baseline: 1.1458x; 1.1458x over previous
"""Block-causal (frame-windowed) attention layer for Trainium2, 8-core SPMD.

Reference computation (B=4, T=2048, C=512, H=8, Dh=64, NPATCH=256):
  LayerNorm(x) -> qkv = xn @ w_qkv -> per-head attention with mask
  frame(i) >= frame(j), frame = idx // 256 -> out @ w_out + b_out

Sharding: core c handles batch c//2 and heads (c%2)*4 .. (c%2)*4+3.
Each core computes a partial y (its heads' contribution to out @ w_out);
the host sums the two partials per batch and adds b_out.

Key design points (v2):
 - frames are 256 wide and the mask is frame-aligned: no masking ops.
 - S^T layout [keys, q]; softmax normalizer via ones-column in V.
 - x is fp16 on the wire; LayerNorm apply on GpSimd; transposes are
   plain PE matmuls vs identity (fast + keeps HAM warm); PE warmup burst.
 - S^T matmuls of a head pair are row-packed (base partitions 0/64) so
   they run concurrently in distinct PE row groups.
 - exp: 3 of 4 heads on ScalarE (exact); head 3 uses a Schraudolph-style
   fp16 exp on VectorE (affine in log2-space, bitcast uint16->fp16).
   The 2^2 exponent offset cancels in the softmax normalizer.
"""

import sys

sys.path.insert(0, "/opt/trn_rl_repo")

import numpy as np

import concourse.bacc as bacc
import concourse.bass as bass
import concourse.mybir as mybir
import concourse.tile as tile
from concourse.bass_utils import run_bass_kernel_spmd
from concourse.masks import make_identity

B, T, C = 4, 2048, 512
HEADS, DH = 8, 64
NPATCH = 256
EPS = 1e-5
N_CORES = 8
HPC = HEADS // 2          # heads per core = 4
QK_COLS = HPC * DH * 2    # 512 (q block + k block)
V_COLS = HPC * DH         # 256
NT = T // 128             # 16 token tiles
NF = T // NPATCH          # 8 frames
NCC = C // 128            # 4 contraction chunks

F32 = mybir.dt.float32
FP16 = mybir.dt.float16
U16 = mybir.dt.uint16
AF = mybir.ActivationFunctionType
ALU = mybir.AluOpType

# Schraudolph fp16 exp: bits = S*1024/ln2 + (15*1024 - 44.7 + 2048)
# (the +2048 = 2^2 scale keeps the affine form inside [0, 32768) for
# S in (-11.7, +10.4); the 4x scale cancels in the softmax normalizer)
EXP_A = 1024.0 / float(np.log(2.0))
EXP_B = 15 * 1024 - 44.7 + 2048.0

_cache = {}
_run_opts = {}      # test harness may set {"trace": True, ...}
_last_res = [None]  # last BassKernelResults, for profiling
import os
_NO_VEXP = bool(os.environ.get("NO_VEXP"))
_DEBUG = bool(os.environ.get("KDEBUG"))
_NO_PACK = bool(os.environ.get("NO_PACK"))    # ST matmuls full-width K=128
_OLD_T = bool(os.environ.get("OLD_T"))        # transpose-mode transposes
_OLD_LN = bool(os.environ.get("OLD_LN"))      # LN apply on vector
_OLD_VEVAC = bool(os.environ.get("OLD_VEVAC"))  # v evac on vector


def _build(with_qkv_bias: bool):
    nc = bacc.Bacc("TRN2", target_bir_lowering=False, debug=False,
                   num_devices=N_CORES)
    x_d = nc.dram_tensor("x", [T, C], FP16, kind="ExternalInput").ap()
    wqk_d = nc.dram_tensor("wqk", [C, QK_COLS], FP16, kind="ExternalInput").ap()
    wv_d = nc.dram_tensor("wv", [C, V_COLS], FP16, kind="ExternalInput").ap()
    wo_d = nc.dram_tensor("wo", [V_COLS, C], FP16, kind="ExternalInput").ap()
    if with_qkv_bias:
        bqk_d = nc.dram_tensor("bqk", [1, QK_COLS], F32, kind="ExternalInput").ap()
        bv_d = nc.dram_tensor("bv", [1, 2 * V_COLS], F32, kind="ExternalInput").ap()
    y_d = nc.dram_tensor("y", [T, C], F32, kind="ExternalOutput").ap()
    dbg = None
    if _DEBUG:
        dbg = {
            "dbg_qkT": nc.dram_tensor("dbg_qkT", [128, NCC * T], FP16,
                                      kind="ExternalOutput").ap(),
            "dbg_v": nc.dram_tensor("dbg_v", [128, NT * HPC * (DH + 1)], FP16,
                                    kind="ExternalOutput").ap(),
            "dbg_oT": nc.dram_tensor("dbg_oT", [128, 2 * T], FP16,
                                     kind="ExternalOutput").ap(),
            "dbg_pv": nc.dram_tensor("dbg_pv", [65, 16 * 512], F32,
                                     kind="ExternalOutput").ap(),
            "dbg_rec": nc.dram_tensor("dbg_rec", [1, 16 * 512], F32,
                                      kind="ExternalOutput").ap(),
        }

    with tile.TileContext(nc) as tc:
        _emit(nc, tc, x_d, wqk_d, wv_d, wo_d, y_d,
              (bqk_d, bv_d) if with_qkv_bias else None, dbg)
    nc.compile()
    return nc


def _emit(nc, tc, x_d, wqk_d, wv_d, wo_d, y_d, biases, dbg=None):
    from contextlib import ExitStack
    ctx = ExitStack()
    with ctx:
        singles = ctx.enter_context(tc.tile_pool(name="singles", bufs=1))
        xnp = ctx.enter_context(tc.tile_pool(name="xnp", bufs=3))
        stats = ctx.enter_context(tc.tile_pool(name="stats", bufs=4))
        ptp = ctx.enter_context(tc.tile_pool(name="ptp", bufs=6))
        recips = ctx.enter_context(tc.tile_pool(name="recips", bufs=2))
        rrepp = ctx.enter_context(tc.tile_pool(name="rrepp", bufs=2))
        yp = ctx.enter_context(tc.tile_pool(name="yp", bufs=3))
        ps_mm = ctx.enter_context(tc.tile_pool(name="ps_mm", bufs=2, space="PSUM"))
        ps_st = ctx.enter_context(tc.tile_pool(name="ps_st", bufs=4, space="PSUM"))
        ps_pv = ctx.enter_context(tc.tile_pool(name="ps_pv", bufs=2, space="PSUM"))

        # ---- identity + constants first (gate the PE warmup burst) ----
        ident = singles.tile([128, 128], FP16)
        make_identity(nc, ident)
        warm_src = singles.tile([128, 512], FP16)
        nc.vector.memset(warm_src, 0.0)

        # ---- input DMA: x in chunks on two queues, weights on scalar ----
        xt_all = singles.tile([128, NT, C], FP16)
        x_r = x_d.rearrange("(n p) c -> p n c", p=128)
        nc.sync.dma_start(out=xt_all[:, 0:2, :], in_=x_r[:, 0:2, :])
        nc.gpsimd.dma_start(out=xt_all[:, 2:5, :], in_=x_r[:, 2:5, :])
        nc.sync.dma_start(out=xt_all[:, 5:10, :], in_=x_r[:, 5:10, :])
        nc.gpsimd.dma_start(out=xt_all[:, 10:16, :], in_=x_r[:, 10:16, :])

        wqk = singles.tile([128, NCC, QK_COLS], FP16)
        wv = singles.tile([128, NCC, V_COLS], FP16)
        wo = singles.tile([128, 2, C], FP16)
        nc.scalar.dma_start(
            out=wqk, in_=wqk_d.rearrange("(cc p) n -> p cc n", p=128))
        nc.scalar.dma_start(
            out=wv, in_=wv_d.rearrange("(cc p) n -> p cc n", p=128))
        nc.scalar.dma_start(
            out=wo, in_=wo_d.rearrange("(i p) n -> p i n", p=128))

        if biases is not None:
            bqk_d, bv_d = biases
            bqk_sb = singles.tile([128, NCC, 1], F32)
            nc.gpsimd.dma_start(
                out=bqk_sb, in_=bqk_d.rearrange("o (d p) -> p d o", p=128))
            # v bias duplicated (two token tiles share one evac): [128, 512]
            bv_sb = singles.tile([128, 2 * V_COLS], F32)
            nc.gpsimd.dma_start(out=bv_sb,
                                in_=bv_d.to_broadcast((128, 2 * V_COLS)))

        # ---- persistent tiles ----
        eps_t = singles.tile([128, 1], F32)
        nc.vector.memset(eps_t, EPS)

        xnT = singles.tile([128, NCC, T], FP16)      # [C dims, (cc, tok)]
        qkT = singles.tile([128, NCC, T], FP16)      # d0,d1 = q; d2,d3 = k
        v_all = singles.tile([128, NT, HPC, DH + 1], FP16)
        oT = singles.tile([128, 2, T], FP16)         # [inner dims, tok]

        ones_stage = singles.tile([128, NT * HPC], F32)
        nc.vector.memset(ones_stage, 1.0)
        nc.vector.tensor_copy(
            out=v_all[:, :, :, DH:DH + 1].rearrange("p t h o -> p (t h o)"),
            in_=ones_stage)

        # ---- PE warmup: dummy matmuls so HAM un-throttles before stage B ----
        for w in range(8):
            pw = ps_st.tile([128, 512], F32, tag="st")
            nc.tensor.matmul(pw, ident, warm_src, start=True, stop=True)

        # ---- stage A: LayerNorm + transpose into xnT ----
        for t in range(NT):
            xt = xt_all[:, t, :]
            st6 = stats.tile([128, 6], F32)
            nc.vector.bn_stats(out=st6, in_=xt)
            mv = stats.tile([128, 2], F32)
            nc.vector.bn_aggr(out=mv, in_=st6)
            rstd = stats.tile([128, 1], F32)
            nc.scalar.activation(out=rstd, in_=mv[:, 1:2], func=AF.Sqrt,
                                 bias=eps_t, scale=1.0)
            nc.vector.reciprocal(out=rstd, in_=rstd)
            xn = xnp.tile([128, C], FP16)
            # NOTE: gpsimd tensor_scalar with AP scalars is ~7.6us/op - keep
            # the LN apply on the vector engine.
            nc.vector.tensor_scalar(
                out=xn, in0=xt, scalar1=mv[:, 0:1], scalar2=rstd,
                op0=ALU.subtract, op1=ALU.mult)
            if _OLD_T:
                tp16 = ps_mm.tile([128, 512], FP16, tag="mm")
                for cc in range(NCC):
                    nc.tensor.transpose(
                        tp16[:, cc * 128:(cc + 1) * 128],
                        xn[:, cc * 128:(cc + 1) * 128], ident)
                nc.vector.tensor_copy(
                    out=xnT[:, :, t * 128:(t + 1) * 128],
                    in_=tp16.rearrange("p (cc q) -> p cc q", cc=NCC))
            else:
                tp = ps_mm.tile([128, 512], F32, tag="mm")
                for cc in range(NCC):
                    nc.tensor.matmul(
                        tp[:, cc * 128:(cc + 1) * 128],
                        xn[:, cc * 128:(cc + 1) * 128], ident,
                        start=True, stop=True)
                nc.vector.tensor_copy(
                    out=xnT[:, :, t * 128:(t + 1) * 128],
                    in_=tp.rearrange("p (cc q) -> p cc q", cc=NCC))

        # ---- stage B: qkT = w_qk^T @ xn^T ; v = xn @ w_v ----
        for n in range(4):           # token groups of 512
            for d in range(NCC):     # qk dim chunks
                mm = ps_mm.tile([128, 512], F32, tag="mm")
                for cc in range(NCC):
                    nc.tensor.matmul(
                        mm,
                        wqk[:, cc, d * 128:(d + 1) * 128],
                        xnT[:, cc, n * 512:(n + 1) * 512],
                        start=(cc == 0), stop=(cc == NCC - 1))
                if biases is not None:
                    nc.vector.tensor_scalar(
                        out=qkT[:, d, n * 512:(n + 1) * 512], in0=mm,
                        scalar1=bqk_sb[:, d, :], scalar2=None,
                        op0=ALU.add)
                else:
                    nc.vector.tensor_copy(
                        out=qkT[:, d, n * 512:(n + 1) * 512], in_=mm)
            for half in range(2):    # two token tiles per PSUM bank
                t0 = 4 * n + 2 * half
                mm = ps_mm.tile([128, 512], F32, tag="mm")
                for sub in range(2):
                    t = t0 + sub
                    for cc in range(NCC):
                        nc.tensor.matmul(
                            mm[:, sub * 256:(sub + 1) * 256],
                            xnT[:, cc, t * 128:(t + 1) * 128],
                            wv[:, cc, :],
                            start=(cc == 0), stop=(cc == NCC - 1))
                v_out = v_all[:, t0:t0 + 2, :, 0:DH]
                v_in = mm.rearrange("p (t h d) -> p t h d", t=2, h=HPC)
                if biases is not None:
                    nc.vector.tensor_tensor(
                        out=v_out, in0=v_in,
                        in1=bv_sb.rearrange("p (t h d) -> p t h d",
                                            t=2, h=HPC),
                        op=ALU.add)
                elif _OLD_VEVAC:
                    nc.vector.tensor_copy(out=v_out, in_=v_in)
                else:
                    nc.scalar.activation(out=v_out, in_=v_in, func=AF.Copy)

        # ---- stage C: attention per (frame, head pair); D: out-proj ----
        for f in range(NF):
            for p in range(2):       # head pair: heads (2p, 2p+1)
                dq, dk = p, 2 + p
                q_a = qkT[0:64, dq, f * 256:(f + 1) * 256]
                q_b = qkT[64:128, dq, f * 256:(f + 1) * 256]
                # both heads share one PSUM bank; a matmul's start=True
                # clears the WHOLE bank's has_written, so only head A's
                # first matmul carries start=True (head B's first write then
                # lands on already-cleared has_written and overwrites)
                pvt = ps_pv.tile([65, 512], F32, tag="pv")
                pv = (pvt[:, 0:256], pvt[:, 256:512])
                pts = []
                for g in range(f + 1):   # source frame (2 key chunks)
                    st_a = ps_st.tile([128, 512], F32, tag="st")
                    st_b = ps_st.tile([128, 512], F32, tag="st")
                    for j in range(2):
                        kc = 2 * g + j
                        ksl = slice(kc * 128, (kc + 1) * 128)
                        csl = slice(j * 256, (j + 1) * 256)
                        if _NO_PACK:
                            nc.tensor.matmul(st_a[:, csl],
                                             qkT[0:64, dk, ksl], q_a,
                                             start=True, stop=True,
                                             tile_position=(0, 0))
                            nc.tensor.matmul(st_b[:, csl],
                                             qkT[64:128, dk, ksl], q_b,
                                             start=True, stop=True,
                                             tile_position=(0, 0))
                        else:
                            nc.tensor.matmul(st_a[:, csl],
                                             qkT[0:64, dk, ksl], q_a,
                                             start=True, stop=True)
                            nc.tensor.matmul(st_b[:, csl],
                                             qkT[64:128, dk, ksl], q_b,
                                             start=True, stop=True)
                    pt_a = ptp.tile([128, 512], FP16)
                    pt_b = ptp.tile([128, 512], FP16)
                    nc.scalar.activation(out=pt_a, in_=st_a, func=AF.Exp)
                    if p == 1 and not _NO_VEXP:
                        # fast exp on DVE: affine in log2 space -> uint16
                        nc.vector.tensor_scalar(
                            out=pt_b.bitcast(U16), in0=st_b,
                            scalar1=EXP_A, scalar2=EXP_B,
                            op0=ALU.mult, op1=ALU.add)
                    else:
                        nc.scalar.activation(out=pt_b, in_=st_b, func=AF.Exp)
                    pts.append((pt_a, pt_b))
                    if g > 0:
                        _emit_pv(nc, pv, v_all, pts[g - 1], p, f, g - 1)
                _emit_pv(nc, pv, v_all, pts[f], p, f, f)
                # normalize: oT = pv[:64] / pv[64]
                ssum = recips.tile([1, 512], F32)
                nc.vector.tensor_copy(out=ssum, in_=pvt[64:65, :])
                rec = recips.tile([1, 512], F32)
                nc.vector.reciprocal_approx_fast(out=rec, in_=ssum)
                if dbg is not None:
                    fp = 2 * f + p
                    pv_sb = singles.tile([65, 16, 512], F32, tag="dbg_pv_sb")
                    nc.vector.tensor_copy(out=pv_sb[:, fp, :], in_=pvt)
                    nc.sync.dma_start(
                        out=dbg["dbg_pv"][:, fp * 512:(fp + 1) * 512],
                        in_=pv_sb[:, fp, :])
                    nc.sync.dma_start(
                        out=dbg["dbg_rec"][:, fp * 512:(fp + 1) * 512],
                        in_=rec)
                rrep = rrepp.tile([64, 512], F32)
                nc.gpsimd.partition_broadcast(rrep, rec)
                fsl = slice(f * 256, (f + 1) * 256)
                nc.vector.tensor_tensor(
                    out=oT[0:64, p, fsl], in0=pvt[0:64, 0:256],
                    in1=rrep[:, 0:256], op=ALU.mult)
                nc.vector.tensor_tensor(
                    out=oT[64:128, p, fsl], in0=pvt[0:64, 256:512],
                    in1=rrep[:, 256:512], op=ALU.mult)
            # out-projection for this frame's two token tiles
            for t in (2 * f, 2 * f + 1):
                ym = ps_mm.tile([128, 512], F32, tag="mm")
                for i in range(2):
                    nc.tensor.matmul(
                        ym, oT[:, i, t * 128:(t + 1) * 128], wo[:, i, :],
                        start=(i == 0), stop=(i == 1))
                ysb = yp.tile([128, C], F32)
                nc.vector.tensor_copy(out=ysb, in_=ym)
                nc.gpsimd.dma_start(
                    out=y_d[t * 128:(t + 1) * 128, :], in_=ysb)
        if dbg is not None:
            nc.sync.dma_start(
                out=dbg["dbg_qkT"],
                in_=qkT.rearrange("p cc t -> p (cc t)"))
            nc.sync.dma_start(
                out=dbg["dbg_v"],
                in_=v_all.rearrange("p t h d -> p (t h d)"))
            nc.sync.dma_start(
                out=dbg["dbg_oT"], in_=oT.rearrange("p i t -> p (i t)"))


def _emit_pv(nc, pv, v_all, pt_pair, p, f, g):
    pv_a, pv_b = pv
    pt_a, pt_b = pt_pair
    last = f
    for j in range(2):
        kc = 2 * g + j
        csl = slice(j * 256, (j + 1) * 256)
        # start=True ONLY on head A's first matmul: it clears the whole
        # bank's has_written, covering head B's region too.
        nc.tensor.matmul(pv_a, v_all[:, kc, 2 * p, :], pt_a[:, csl],
                         start=(kc == 0), stop=(kc == 2 * last + 1),
                         skip_group_check=True)
        nc.tensor.matmul(pv_b, v_all[:, kc, 2 * p + 1, :], pt_b[:, csl],
                         start=False, stop=(kc == 2 * last + 1),
                         skip_group_check=True)


def kernel(x, ln_gamma, ln_beta, w_qkv, w_out, b_out, mask):
    x = np.asarray(x, dtype=np.float32)
    ln_gamma = np.asarray(ln_gamma, dtype=np.float32)
    ln_beta = np.asarray(ln_beta, dtype=np.float32)
    w_qkv = np.asarray(w_qkv, dtype=np.float32)
    w_out = np.asarray(w_out, dtype=np.float32)
    b_out = np.asarray(b_out, dtype=np.float32)

    inner = HEADS * DH
    wq_all = w_qkv[:, 0:inner] * ln_gamma[:, None]
    wk_all = w_qkv[:, inner:2 * inner] * ln_gamma[:, None]
    wv_all = w_qkv[:, 2 * inner:3 * inner] * ln_gamma[:, None]
    scale = DH ** -0.5
    # beta contribution to q/k/v (exact: qkv = ln(x)@(gamma*W) + beta@W)
    bq_all = ln_beta @ w_qkv[:, 0:inner]
    bk_all = ln_beta @ w_qkv[:, inner:2 * inner]
    bv_all = ln_beta @ w_qkv[:, 2 * inner:3 * inner]
    with_bias = bool(
        np.abs(bq_all).max() > 0 or np.abs(bk_all).max() > 0
        or np.abs(bv_all).max() > 0)

    key = ("prog", with_bias, _NO_VEXP, _NO_PACK, _OLD_T, _OLD_LN, _OLD_VEVAC,
           _DEBUG)
    if key not in _cache:
        _cache[key] = _build(with_bias)
    nc = _cache[key]

    in_maps = []
    for c in range(N_CORES):
        b = c // 2
        h0 = (c % 2) * HPC
        cols = slice(h0 * DH, (h0 + HPC) * DH)
        wqk_c = np.concatenate([wq_all[:, cols] * scale, wk_all[:, cols]],
                               axis=1)
        m = {
            "x": np.ascontiguousarray(x[b].astype(np.float16)),
            "wqk": np.ascontiguousarray(wqk_c.astype(np.float16)),
            "wv": np.ascontiguousarray(wv_all[:, cols].astype(np.float16)),
            "wo": np.ascontiguousarray(w_out[cols, :].astype(np.float16)),
        }
        if with_bias:
            bqk_c = np.concatenate([bq_all[cols] * scale, bk_all[cols]])
            m["bqk"] = np.ascontiguousarray(bqk_c[None, :])
            m["bv"] = np.ascontiguousarray(
                np.tile(bv_all[cols], 2)[None, :])
        in_maps.append(m)

    res = run_bass_kernel_spmd(nc, in_maps, core_ids=list(range(N_CORES)),
                               **_run_opts)
    _last_res[0] = res
    y = np.empty((B, T, C), dtype=np.float32)
    for b in range(B):
        y[b] = (res.results[2 * b]["y"].astype(np.float32)
                + res.results[2 * b + 1]["y"].astype(np.float32) + b_out)
    return y
